# revision 12
# baseline (speedup 1.0000x reference)
"""Trainium2 Bass kernel for nn_NeuralClustering (segment_reduce).

Strategy: data-parallel over B across 8 NeuronCores (32 batches/core),
MLP weights replicated. The h-encoder runs feature-major on the TensorE.
The final encoder layer (W5) is algebraically commuted past the segment
sum: H = W5^T (a4 @ onehot) + counts * b5, so the [B,N,256] `hs` tensor
is never materialized. Only tokens < n (which need per-cluster sums) are
emitted token-major in the 4th hidden layer; for tokens >= n the Q
(unclustered) sum falls out of the ScalarE activation's accum_out and
hn (= hs[:, n]) is column 0 of the feature-major tile. The g-MLP, G
recombination, and E-MLP then run on small feature-major blocks.
"""
import os
import numpy as np

import concourse.bass as bass
import concourse.mybir as mybir
import concourse.tile as tile
from concourse import bacc
from concourse.bass_utils import run_bass_kernel_spmd

F32 = mybir.dt.float32
I32 = mybir.dt.int32
ALU = mybir.AluOpType
ACTF = mybir.ActivationFunctionType

NCORES = 8
ST = 100            # tokens per subtile (token-major partition dim)

_prog_cache = {}
LAST_RESULT = [None]  # run_bass_kernel_spmd result of the last call


def _ap(t, pattern, extra_off=0):
    return bass.AP(tensor=t.tensor, offset=t.offset + extra_off, ap=pattern)


def _flat(t):
    """Flatten all free dims of a (contiguous) tile into one."""
    n = 1
    for step, cnt in t.ap[1:]:
        n *= cnt
    return bass.AP(tensor=t.tensor, offset=t.offset,
                   ap=[list(t.ap[0]), [1, n]])


def _build(B_c, N, n, K, alphas):
    """Build + compile the per-core program. alphas = (ah(4), ag(5), ae(5))."""
    SLOT = K + 2          # K clusters, Q, hn
    R17 = 2 * K + 1       # Hall rows per batch
    KP1 = K + 1
    TW = N // 2           # 400 tokens per half-batch tile
    SPT = TW // ST        # token-major subtiles in the clustered half = 4
    RALL = B_c * R17      # 544 g-MLP rows
    RH = RALL // 2        # 272 per N-half
    ER = B_c * KP1        # 288 E-MLP rows
    ah, ag, ae = alphas
    assert n == TW and TW % ST == 0

    nc = bacc.Bacc("TRN2", target_bir_lowering=False, debug=False,
                   num_devices=NCORES)

    def din(name, shape, dt=F32):
        return nc.dram_tensor(name, list(shape), dt, kind="ExternalInput").ap()

    data_d = din("data_t", [B_c, 2, N])
    cs_d = din("cs_tok", [ST, B_c, TW // ST], I32)
    counts_d = din("counts", [B_c, SLOT])
    maskk_d = din("maskk", [B_c, K])
    amask_d = din("amask", [B_c, KP1])
    bmask_d = din("bmask", [B_c, KP1])
    gsmask_d = din("gsmask", [B_c, R17])
    hw_d = [din(f"hW{i}", s) for i, s in
            enumerate([[2, 128], [128, 128], [128, 128], [128, 128],
                       [128, 256]], 1)]
    hb_d = [din(f"hb{i}", [128]) for i in range(1, 5)]
    hb4x4_d = din("hb4x4", [512])
    hb5pp_d = din("hb5pp", [128, 2])
    gw1_d = din("gW1", [128, 2, 128])
    gw_d = [din(f"gW{i}", [128, 128]) for i in range(2, 6)]
    gw6_d = din("gW6", [128, 512])
    gb_d = [din(f"gb{i}", [128]) for i in range(1, 6)]
    gb6pp_d = din("gb6pp", [128, 4])
    ew1_d = din("eW1", [128, 6, 128])
    ew_d = [din(f"eW{i}", [128, 128]) for i in range(2, 6)]
    ew6_d = din("eW6", [128, 1])
    eb_d = [din(f"eb{i}", [128]) for i in range(1, 6)]
    eb6_d = din("eb6", [1])
    e_out = nc.dram_tensor("E_out", [B_c, KP1], F32,
                           kind="ExternalOutput").ap()

    with tile.TileContext(nc) as tc:
        with (
            tc.tile_pool(name="wp", bufs=1) as wp,
            tc.tile_pool(name="persist", bufs=1) as pp,
        ):
            _uid = [0]

            def load(dram_ap, shape, dt=F32, eng=None):
                _uid[0] += 1
                nm = f"w{_uid[0]}_{dram_ap.tensor.name}"
                t = wp.tile(list(shape), dt, name=nm, tag=nm)
                (eng or nc.sync).dma_start(out=t, in_=dram_ap)
                return t

            def loadb(dram_ap, width, eng=None):
                _uid[0] += 1
                nm = f"b{_uid[0]}_{dram_ap.tensor.name}"
                t = wp.tile([128, width], F32, name=nm, tag=nm)
                (eng or nc.sync).dma_start(
                    out=t, in_=_ap(dram_ap, [[0, 128], [1, width]]))
                return t

            # ---- phase-A weights/constants (sync queue, issued first) ----
            w1_t = load(hw_d[0], [2, 128])
            w2_t = load(hw_d[1], [128, 128])
            w3_t = load(hw_d[2], [128, 128])
            w4_t = load(hw_d[3], [128, 128])
            b1_t = load(hb_d[0][:, None], [128, 1])
            b2_t = load(hb_d[1][:, None], [128, 1])
            b3_t = load(hb_d[2][:, None], [128, 1])
            b4_t = load(hb_d[3][:, None], [128, 1])
            b4r_t = load(hb4x4_d[None, :], [1, 512])

            ones_t = wp.tile([1, ST], F32)
            nc.vector.memset(ones_t, 1.0)
            iota_t = wp.tile([128, K], I32)
            nc.gpsimd.iota(iota_t, pattern=[[1, K]], base=0,
                           channel_multiplier=0)

            # one-hot for clustered half: [ST, B_c, SPT, K]
            cs_tok = wp.tile([ST, B_c, SPT], I32)
            nc.sync.dma_start(out=cs_tok, in_=cs_d)
            oh_all = pp.tile([ST, B_c, SPT, K], F32)
            nc.vector.tensor_tensor(
                out=oh_all,
                in0=_ap(iota_t, [[iota_t.ap[0][0], ST], [0, B_c], [0, SPT],
                                 [1, K]]),
                in1=_ap(cs_tok, [[cs_tok.ap[0][0], ST], [SPT, B_c],
                                 [1, SPT], [0, K]]),
                op=ALU.is_equal)

            C_all = pp.tile([128, B_c, SLOT], F32)

            # ---- phase A: encoder + segment reduce ----
            with (
                tc.tile_pool(name="xp", bufs=2) as xp,
                tc.tile_pool(name="ap_", bufs=2) as ap_,
                tc.tile_pool(name="a4p", bufs=3) as a4p,
                tc.tile_pool(name="pap", bufs=3, space="PSUM") as pap,
                tc.tile_pool(name="p4p", bufs=2, space="PSUM") as p4p,
                tc.tile_pool(name="pcp", bufs=1, space="PSUM") as pcp,
            ):
                for b in range(B_c):
                    x_t = xp.tile([2, N], F32)
                    nc.sync.dma_start(out=x_t, in_=data_d[b])
                    ps_c = pcp.tile([128, K], F32)
                    for h in range(2):
                        tsl = slice(h * TW, (h + 1) * TW)
                        ps1 = pap.tile([128, TW], F32, tag="psa")
                        nc.tensor.matmul(ps1, w1_t, x_t[:, tsl],
                                         start=True, stop=True)
                        a1 = ap_.tile([128, TW], F32, tag="a1")
                        nc.scalar.activation(out=a1, in_=ps1, func=ACTF.Prelu,
                                             bias=b1_t, scale=1.0, alpha=ah[0])
                        ps2 = pap.tile([128, TW], F32, tag="psa")
                        nc.tensor.matmul(ps2, w2_t, a1, start=True, stop=True)
                        a2 = ap_.tile([128, TW], F32, tag="a2")
                        nc.scalar.activation(out=a2, in_=ps2, func=ACTF.Prelu,
                                             bias=b2_t, scale=1.0, alpha=ah[1])
                        ps3 = pap.tile([128, TW], F32, tag="psa")
                        nc.tensor.matmul(ps3, w3_t, a2, start=True, stop=True)
                        # L3 PReLU on DVE (2 ops) to offload ACT
                        a3t = ap_.tile([128, TW], F32, tag="a3t")
                        nc.vector.tensor_scalar(out=a3t, in0=ps3, scalar1=b3_t,
                                                scalar2=None, op0=ALU.add)
                        a3 = ap_.tile([128, TW], F32, tag="a3")
                        nc.vector.scalar_tensor_tensor(
                            out=a3, in0=a3t, scalar=ah[2], in1=a3t,
                            op0=ALU.mult, op1=ALU.max)
                        if h == 0:
                            # clustered half: token-major L4, bias via K=1 MM
                            ps4 = p4p.tile([ST, 512], F32, tag="p4tok")
                            nc.tensor.matmul(ps4, ones_t, b4r_t,
                                             start=True, stop=False)
                            for s in range(SPT):
                                nc.tensor.matmul(
                                    ps4[:, s * 128:(s + 1) * 128],
                                    a3[:, s * ST:(s + 1) * ST], w4_t,
                                    start=False, stop=(s == SPT - 1))
                            a4 = a4p.tile([ST, 512], F32, tag="a4")
                            nc.scalar.activation(out=a4, in_=ps4,
                                                 func=ACTF.Prelu, bias=0.0,
                                                 scale=1.0, alpha=ah[3])
                            for s in range(SPT):
                                nc.tensor.matmul(
                                    ps_c, a4[:, s * 128:(s + 1) * 128],
                                    oh_all[:, b, s, :],
                                    start=(s == 0), stop=(s == SPT - 1))
                        else:
                            # unclustered half: feature-major L4,
                            # Q := row-sum via accum_out, hn := column 0
                            ps4f = p4p.tile([128, TW], F32, tag="p4f")
                            nc.tensor.matmul(ps4f, w4_t, a3,
                                             start=True, stop=True)
                            a4f = a4p.tile([128, TW], F32, tag="a4f")
                            qp = a4p.tile([128, 1], F32, tag="qp")
                            nc.scalar.activation(out=a4f, in_=ps4f,
                                                 func=ACTF.Prelu, bias=b4_t,
                                                 scale=1.0, alpha=ah[3],
                                                 accum_out=qp)
                            nc.gpsimd.tensor_copy(C_all[:, b, K:K + 1], qp)
                            nc.gpsimd.tensor_copy(C_all[:, b, K + 1:K + 2],
                                                  a4f[:, 0:1])
                    nc.vector.tensor_copy(C_all[:, b, 0:K], ps_c)

            # ---- phase-B weights/constants (gpsimd queue, after phase A
            # in program order so they don't block phase-A DMAs) ----
            gq = nc.gpsimd
            w5_t = load(hw_d[4], [128, 256], eng=gq)
            b5pp_t = load(hb5pp_d, [128, 2], eng=gq)
            gw1_t = load(gw1_d, [128, 2, 128], eng=gq)
            gw_t = [load(d, [128, 128], eng=gq) for d in gw_d]
            gw6_t = load(gw6_d, [128, 512], eng=gq)
            gb_t = [load(d[:, None], [128, 1], eng=gq) for d in gb_d]
            gb6pp_t = load(gb6pp_d, [128, 4], eng=gq)
            ew1_t = load(ew1_d, [128, 6, 128], eng=gq)
            ew_t = [load(d, [128, 128], eng=gq) for d in ew_d]
            ew6_t = load(ew6_d, [128, 1], eng=gq)
            eb_t = [load(d[:, None], [128, 1], eng=gq) for d in eb_d]
            eb6_t = load(eb6_d[None, :], [1, 1], eng=gq)
            counts_bc = loadb(counts_d, B_c * SLOT, eng=gq)
            maskk_bc = loadb(maskk_d, B_c * K, eng=gq)
            amask_bc = loadb(amask_d, B_c * KP1, eng=gq)
            bmask_bc = loadb(bmask_d, B_c * KP1, eng=gq)
            gsmask_bc = loadb(gsmask_d, B_c * R17, eng=gq)

            # ---- phase B ----
            H_all = []
            with (
                tc.tile_pool(name="bsb", bufs=1) as bp,
                tc.tile_pool(name="phb", bufs=1, space="PSUM") as phb,
                tc.tile_pool(name="pgb", bufs=2, space="PSUM") as pgb,
                tc.tile_pool(name="pg6", bufs=2, space="PSUM") as pg6,
                tc.tile_pool(name="peb", bufs=2, space="PSUM") as peb,
                tc.tile_pool(name="pE", bufs=1, space="PSUM") as pEp,
            ):
                # H = W5^T C + counts * b5   -> [128, B_c, SLOT] x2 chunks
                for c in range(2):
                    psH = phb.tile([128, B_c * SLOT], F32, tag="psH")
                    nc.tensor.matmul(psH, w5_t[:, c * 128:(c + 1) * 128],
                                     _flat(C_all), start=True, stop=True)
                    Hc = bp.tile([128, B_c, SLOT], F32, tag=f"H{c}",
                                 name=f"H{c}")
                    nc.vector.scalar_tensor_tensor(
                        out=_flat(Hc), in0=counts_bc,
                        scalar=b5pp_t[:, c:c + 1], in1=psH,
                        op0=ALU.mult, op1=ALU.add)
                    H_all.append(Hc)

                # Hall [128, B_c, R17] x2  (copies on Pool, add on DVE)
                Hall = []
                for c in range(2):
                    Hc = H_all[c]
                    ha = bp.tile([128, B_c, R17], F32, tag=f"Hall{c}",
                                 name=f"Hall{c}")
                    nc.gpsimd.tensor_copy(ha[:, :, 0:K], Hc[:, :, 0:K])
                    nc.vector.tensor_tensor(
                        out=ha[:, :, K:2 * K], in0=Hc[:, :, 0:K],
                        in1=_ap(Hc, [[Hc.ap[0][0], 128], [SLOT, B_c], [0, K]],
                                extra_off=SLOT - 1),
                        op=ALU.add)
                    nc.gpsimd.tensor_copy(ha[:, :, 2 * K:2 * K + 1],
                                          Hc[:, :, SLOT - 1:SLOT])
                    Hall.append(ha)

                # g-MLP over RALL rows in two halves
                gs = [bp.tile([128, B_c, R17], F32, tag=f"gs{mc}",
                              name=f"gs{mc}") for mc in range(4)]
                for nh in range(2):
                    rsl = slice(nh * RH, (nh + 1) * RH)
                    psg = pgb.tile([128, RH], F32, tag="psg")
                    nc.tensor.matmul(psg, gw1_t[:, 0, :],
                                     _flat(Hall[0])[:, rsl],
                                     start=True, stop=False)
                    nc.tensor.matmul(psg, gw1_t[:, 1, :],
                                     _flat(Hall[1])[:, rsl],
                                     start=False, stop=True)
                    act = bp.tile([128, RH], F32, tag="gact")
                    nc.scalar.activation(out=act, in_=psg, func=ACTF.Prelu,
                                         bias=gb_t[0], scale=1.0, alpha=ag[0])
                    for li in range(4):
                        psn = pgb.tile([128, RH], F32, tag="psg")
                        nc.tensor.matmul(psn, gw_t[li], act,
                                         start=True, stop=True)
                        act = bp.tile([128, RH], F32, tag="gact")
                        nc.scalar.activation(out=act, in_=psn, func=ACTF.Prelu,
                                             bias=gb_t[li + 1], scale=1.0,
                                             alpha=ag[li + 1])
                    for mc in range(4):
                        ps6 = pg6.tile([128, RH], F32, tag="ps6")
                        nc.tensor.matmul(ps6, gw6_t[:, mc * 128:(mc + 1) * 128],
                                         act, start=True, stop=True)
                        nc.vector.scalar_tensor_tensor(
                            out=_flat(gs[mc])[:, rsl], in0=ps6,
                            scalar=gb6pp_t[:, mc:mc + 1],
                            in1=gsmask_bc[:, rsl],
                            op0=ALU.add, op1=ALU.mult)

                # G recombination per g-chunk; chunks 0-1 on DVE, 2-3 on Pool
                G3 = []
                for mc in range(4):
                    eng = nc.vector if mc < 2 else nc.gpsimd
                    v = gs[mc]
                    sumK = bp.tile([128, B_c], F32, tag="sumK",
                                   name=f"sumK{mc}")
                    nc.vector.tensor_reduce(out=sumK, in_=v[:, :, 0:K],
                                            op=ALU.add,
                                            axis=mybir.AxisListType.X)
                    G2 = bp.tile([128, B_c, KP1], F32, tag="G2",
                                 name=f"G2_{mc}")
                    t1 = bp.tile([128, B_c, K], F32, tag="t1", name=f"t1_{mc}")
                    eng.tensor_tensor(out=t1, in0=v[:, :, K:2 * K],
                                      in1=v[:, :, 0:K], op=ALU.subtract)
                    t2 = bp.tile([128, B_c, K], F32, tag="t2", name=f"t2_{mc}")
                    eng.tensor_tensor(
                        out=t2, in0=t1,
                        in1=_ap(sumK, [[sumK.ap[0][0], 128], [1, B_c], [0, K]]),
                        op=ALU.add)
                    eng.tensor_tensor(
                        out=G2[:, :, 0:K], in0=t2,
                        in1=_ap(maskk_bc, [[maskk_bc.ap[0][0], 128], [K, B_c],
                                           [1, K]]),
                        op=ALU.mult)
                    eng.tensor_tensor(out=G2[:, :, K:KP1],
                                      in0=sumK[:, :, None],
                                      in1=v[:, :, 2 * K:2 * K + 1],
                                      op=ALU.add)
                    # permute: G3 = G2*A + GK*B
                    tB = bp.tile([128, B_c, KP1], F32, tag="tB",
                                 name=f"tB_{mc}")
                    eng.tensor_tensor(
                        out=tB,
                        in0=_ap(G2, [[G2.ap[0][0], 128], [KP1, B_c], [0, KP1]],
                                extra_off=K),
                        in1=_ap(bmask_bc, [[bmask_bc.ap[0][0], 128],
                                           [KP1, B_c], [1, KP1]]),
                        op=ALU.mult)
                    tA = bp.tile([128, B_c, KP1], F32, tag="tA",
                                 name=f"tA_{mc}")
                    eng.tensor_tensor(
                        out=tA, in0=G2,
                        in1=_ap(amask_bc, [[amask_bc.ap[0][0], 128],
                                           [KP1, B_c], [1, KP1]]),
                        op=ALU.mult)
                    g3 = bp.tile([128, B_c, KP1], F32, tag=f"G3_{mc}",
                                 name=f"G3_{mc}")
                    eng.tensor_tensor(out=g3, in0=tA, in1=tB, op=ALU.add)
                    G3.append(g3)

                # Q broadcast [128, B_c, KP1] x2 (Pool)
                QB = []
                for c in range(2):
                    Hc = H_all[c]
                    qb = bp.tile([128, B_c, KP1], F32, tag=f"QB{c}",
                                 name=f"QB{c}")
                    nc.gpsimd.tensor_copy(
                        qb,
                        _ap(Hc, [[Hc.ap[0][0], 128], [SLOT, B_c], [0, KP1]],
                            extra_off=K))
                    QB.append(qb)

                # E-MLP
                pse = peb.tile([128, ER], F32, tag="pse")
                for j in range(4):
                    nc.tensor.matmul(pse, ew1_t[:, j, :], _flat(G3[j]),
                                     start=(j == 0), stop=False)
                for c in range(2):
                    nc.tensor.matmul(pse, ew1_t[:, 4 + c, :], _flat(QB[c]),
                                     start=False, stop=(c == 1))
                act = bp.tile([128, ER], F32, tag="eact")
                nc.scalar.activation(out=act, in_=pse, func=ACTF.Prelu,
                                     bias=eb_t[0], scale=1.0, alpha=ae[0])
                for li in range(4):
                    psn = peb.tile([128, ER], F32, tag="pse")
                    nc.tensor.matmul(psn, ew_t[li], act, start=True, stop=True)
                    act = bp.tile([128, ER], F32, tag="eact")
                    nc.scalar.activation(out=act, in_=psn, func=ACTF.Prelu,
                                         bias=eb_t[li + 1], scale=1.0,
                                         alpha=ae[li + 1])
                psE = pEp.tile([1, ER], F32)
                nc.tensor.matmul(psE, ew6_t, act, start=True, stop=True)
                E_sb = bp.tile([1, ER], F32, tag="Esb")
                nc.vector.tensor_scalar(out=E_sb, in0=psE,
                                        scalar1=eb6_t[0:1, 0:1], scalar2=None,
                                        op0=ALU.add)
                nc.sync.dma_start(
                    out=_ap(e_out, [[0, 1], [1, ER]]), in_=E_sb)

    nc.compile()
    return nc


def kernel(data, cs, n, h_params, g_params, E_params):
    data = np.asarray(data, np.float32)
    cs = np.asarray(cs, np.int32)
    n = int(n)
    B, N, _ = data.shape
    B_c = B // NCORES

    csn = cs[:, :n]
    K = int(csn.max()) + 1
    SLOT, R17, KP1 = K + 2, 2 * K + 1, K + 1

    hW = [np.asarray(w, np.float32) for w in h_params["W"]]
    hb = [np.asarray(b, np.float32) for b in h_params["b"]]
    ah = [float(a) for a in h_params["a"]]
    gW = [np.asarray(w, np.float32) for w in g_params["W"]]
    gb = [np.asarray(b, np.float32) for b in g_params["b"]]
    ag = [float(a) for a in g_params["a"]]
    eW = [np.asarray(w, np.float32) for w in E_params["W"]]
    eb = [np.asarray(b, np.float32) for b in E_params["b"]]
    ae = [float(a) for a in E_params["a"]]

    # ---- host prep ----
    data_t = np.ascontiguousarray(data.transpose(0, 2, 1))      # [B, 2, N]
    cs2 = np.where(np.arange(N)[None, :] < n, cs, K).astype(np.int32)
    # clustered-half cluster ids, token-major: [ST, B, n//ST]
    cs_tok = np.ascontiguousarray(
        cs2[:, :n].reshape(B, n // ST, ST).transpose(2, 0, 1))
    counts = np.zeros((B, SLOT), np.float32)
    for k in range(K + 1):
        counts[:, k] = (cs2 == k).sum(1)
    counts[:, SLOT - 1] = 1.0
    maskk = (counts[:, :K] > 0).astype(np.float32)
    Ks = csn.max(1)                                             # [B]
    mv = (Ks < K - 1).astype(np.float32)
    pos = np.arange(KP1)[None, :]
    is_new = (pos == (Ks + 1)[:, None]).astype(np.float32)
    amask = 1.0 - is_new * mv[:, None]
    amask[:, K] = 1.0 - mv
    bmask = is_new * mv[:, None]
    bmask[:, K] = 0.0
    gsmask = np.concatenate([maskk, maskk, np.ones((B, 1), np.float32)], 1)
    G_mask = (pos <= (Ks + 1)[:, None]).astype(np.float32)

    key = (B_c, N, n, K, tuple(ah), tuple(ag), tuple(ae))
    if key not in _prog_cache:
        _prog_cache[key] = _build(B_c, N, n, K, (ah, ag, ae))
    nc = _prog_cache[key]

    shared = {
        "hW1": hW[0], "hW2": hW[1], "hW3": hW[2], "hW4": hW[3], "hW5": hW[4],
        "hb1": hb[0], "hb2": hb[1], "hb3": hb[2], "hb4": hb[3],
        "hb4x4": np.tile(hb[3], 4).astype(np.float32),
        "hb5pp": np.ascontiguousarray(hb[4].reshape(2, 128).T),
        "gW1": np.ascontiguousarray(
            np.stack([gW[0][0:128], gW[0][128:256]], 1)),
        "gW2": gW[1], "gW3": gW[2], "gW4": gW[3], "gW5": gW[4],
        "gW6": gW[5],
        "gb1": gb[0], "gb2": gb[1], "gb3": gb[2], "gb4": gb[3], "gb5": gb[4],
        "gb6pp": np.ascontiguousarray(gb[5].reshape(4, 128).T),
        "eW1": np.ascontiguousarray(
            np.stack([eW[0][j * 128:(j + 1) * 128] for j in range(6)], 1)),
        "eW2": eW[1], "eW3": eW[2], "eW4": eW[3], "eW5": eW[4],
        "eW6": eW[5],
        "eb1": eb[0], "eb2": eb[1], "eb3": eb[2], "eb4": eb[3], "eb5": eb[4],
        "eb6": eb[5],
    }
    in_maps = []
    for i in range(NCORES):
        sl = slice(i * B_c, (i + 1) * B_c)
        m = dict(shared)
        m.update({
            "data_t": data_t[sl], "cs_tok": cs_tok[:, sl],
            "counts": counts[sl], "maskk": maskk[sl], "amask": amask[sl],
            "bmask": bmask[sl], "gsmask": gsmask[sl],
        })
        in_maps.append(m)

    res = run_bass_kernel_spmd(nc, in_maps, list(range(NCORES)),
                               trace=bool(os.environ.get("BASS_TRACE")))
    LAST_RESULT[0] = res
    E = np.concatenate([r["E_out"] for r in res.results], 0)
    return E, G_mask
